# revision 3
# baseline (speedup 1.0000x reference)
"""Trainium2 Bass kernel for multi-head attention (Llama-style, GQA 32q/8kv,
RoPE, non-causal softmax as in the source module) distributed over 8
NeuronCores — token-sharded attention, early K/V AllGather.

Distribution (per core c):
  W1: project K,V for the core's kv head over ALL 4096 tokens (PE
      ~110us). kT [hd, t] (RoPE'd) and natural v [t, hd] go to a local
      DRAM staging tile per batch; a Shared-output AllGather per batch
      (1MB in -> 8MB shared out) issues at ~55/110us — first consumed
      ~300us later, so collective latency is fully hidden.
  W2: project Q for ALL 32 heads but only the core's OWN 512 tokens
      (256-token block of each batch, per-core param xTq), RoPE'd into
      SBUF-resident qT[h]. The last DEFER 2-head groups are withheld
      and fed as PE filler inside C(b0).
  C:  attention per (batch-block, kv-head-group): scores -> exp on
      ScalarE -> P@V accumulated in PSUM; denominators on DVE + GPSIMD
      partition reduce; normalized oT[(b,h)] [hd, 256] stays IN SBUF
      (no post-attention exchange: this core owns these tokens
      end-to-end). The attention pipeline is ACT(exp)-bound, so ~190ns
      of PE filler per score-group is popped from a queue: deferred
      Q-projection pieces during C(b0), D o_proj pieces during C(b1).
  D:  o_proj on own 512 tokens with full (streamed) wo, contracting the
      64 SBUF-resident oT tiles; out rows 0:256 = batch-0 block,
      256:512 = batch-1 block.

All matmuls bf16 with fp32 PSUM. RoPE even/odd pairs become contiguous
64-partition blocks via host-side column permutation of wq/wk.
"""

import math
from collections import deque
from contextlib import ExitStack
from dataclasses import dataclass

import numpy as np
import ml_dtypes

import concourse.bass as bass
import concourse.bass_isa as bass_isa
import concourse.mybir as mybir
import concourse.tile as tile
from concourse import bacc

BF16 = mybir.dt.bfloat16
F32 = mybir.dt.float32
AF = mybir.ActivationFunctionType


@dataclass(frozen=True)
class Cfg:
    B: int = 2
    T: int = 2048          # sequence length (per batch)
    D: int = 4096          # model dim
    H: int = 32            # query heads
    HKV: int = 8           # kv heads
    HD: int = 128          # head dim (must be 128)
    NC: int = 8            # cores
    PCH: int = 256         # token chunk for K/V projection (W1)
    SGRP: int = 4          # s-tiles (128) per scores psum group
    DEFER: int = 3         # trailing 2-head wq groups fed as C(b0) filler
    DF_DCH: int = 10       # leading o_proj (b=0) dch chunks fed as C(b1) filler
    QRATE: float = 0.55    # filler pieces per score-group slot in C(b0)
    DRATE: float = 0.85    # filler pieces per score-group slot in C(b1)

    @property
    def TOK(self):
        return self.B * self.T

    @property
    def KD(self):
        return self.D // 128        # contraction tiles over D

    @property
    def TB(self):
        return self.T // self.NC    # per-batch token block per core (256)

    @property
    def QTOK(self):
        return self.B * self.TB     # own tokens (512)

    @property
    def NST(self):
        return self.T // 128        # s-tiles per batch (16)

    @property
    def HG(self):
        return self.H // 2          # wq head groups of 2 (16)


import os as _os

_over = {}
for _k, _cast in (("QRATE", float), ("DRATE", float), ("DEFER", int),
                  ("DF_DCH", int), ("SGRP", int), ("PCH", int)):
    _v = _os.environ.get("KV2_" + _k)
    if _v is not None:
        _over[_k] = _cast(_v)
FULL = Cfg(**_over)


class Fill:
    """Queue of small PE-work emission thunks, metered by a pop rate."""

    def __init__(self, rate=1.0, credit=0.0):
        self.q = deque()
        self.src = deque()
        self.rate = rate
        self.credit = credit

    def add_factory(self, f):
        self.src.append(f)

    def _emit_one(self):
        while not self.q and self.src:
            self.q.extend(self.src.popleft()())
        if self.q:
            self.q.popleft()()
            return True
        return False

    def pop(self):
        self.credit += self.rate
        while self.credit >= 1.0:
            self.credit -= 1.0
            if not self._emit_one():
                return

    def flush(self):
        while self._emit_one():
            pass


def build_nc(cfg: Cfg = FULL, collective: bool = True) -> bass.Bass:
    """Build the SPMD per-core Bass program (identical on all cores).

    collective=False replaces each AllGather with a local DMA of the
    core's own slice (wrong results; TimelineSim profiling only).
    """
    B, T, D, HD, NC = cfg.B, cfg.T, cfg.D, cfg.HD, cfg.NC
    KD, PCH, TB, QTOK = cfg.KD, cfg.PCH, cfg.TB, cfg.QTOK
    SGRP, NST, HG = cfg.SGRP, cfg.NST, cfg.HG
    H = cfg.H
    assert HD == 128 and D % 128 == 0 and T % PCH == 0
    assert NST % SGRP == 0
    NSG = NST // SGRP               # score groups per (b, h)
    KVE = T * HD                    # elems in one batch's kT (= v) part

    nc = bacc.Bacc(
        "TRN2",
        target_bir_lowering=False,
        debug=False,
        num_devices=NC,
    )

    # ---- kernel I/O (per core) ----
    # all weight/activation params are host pre-tiled to [grp][128 p][ko][cols]
    # so every DMA line is one contiguous 8-16KB run per partition
    xTq = nc.declare_dram_parameter("xTq", [128, KD, QTOK], BF16,
                                    isOutput=False)
    wq = nc.declare_dram_parameter("wq", [HG, 128, KD, 2 * HD], BF16,
                                   isOutput=False)
    wk = nc.declare_dram_parameter("wk", [cfg.HKV, 128, KD, HD], BF16,
                                   isOutput=False)
    wv = nc.declare_dram_parameter("wv", [cfg.HKV, 128, KD, HD], BF16,
                                   isOutput=False)
    wo = nc.declare_dram_parameter("wo", [D // 256, 128, H, 256], BF16,
                                   isOutput=False)
    cosq = nc.declare_dram_parameter("cosq", [64, QTOK], F32, isOutput=False)
    sinq = nc.declare_dram_parameter("sinq", [64, QTOK], F32, isOutput=False)
    out = nc.declare_dram_parameter("out", [QTOK, D], F32, isOutput=True)


    scale = 1.0 / math.sqrt(HD)

    with ExitStack() as ctx:
        tc = ctx.enter_context(tile.TileContext(nc))
        dram = ctx.enter_context(tc.tile_pool(name="dram", bufs=1, space="DRAM"))

        # K/V staging + gather buffers; per-core staging covers the
        # core's OWN 512 tokens for ALL 8 kv heads (flat bf16):
        #   [0 : KVO)        kT  viewed [hkv, 128 hd, 512 t]
        #   [KVO : 2*KVO)    v   viewed [hkv, 512 t, 128 hd]
        KVO = cfg.HKV * QTOK * HD
        ag_in = dram.tile([2 * KVO], BF16, name="agin")
        ag_out = dram.tile([NC, 2 * KVO], BF16, addr_space="Shared",
                           name="agout")
        agin_k = ag_in[0:KVO].rearrange("(h p t) -> h p t", h=cfg.HKV, p=128)
        agin_v = ag_in[KVO : 2 * KVO].rearrange("(h t p) -> h t p",
                                                h=cfg.HKV, p=128)

        # persistent SBUF: qT per head, oT per (block, head)
        qkv = ctx.enter_context(tc.tile_pool(name="qkv", bufs=1))
        qT_sb = [qkv.tile([128, QTOK], BF16, name=f"qT{h}") for h in range(H)]
        oT_sb = [
            [qkv.tile([128, TB], BF16, name=f"oT{b}_{h}") for h in range(B * 0 + H)]
            for b in range(B)
        ]

        def rope_apply(rope_pool, dst, psum, c, s, n):
            """psum [128, n] fp32 (evens on parts 0:64, odds 64:128) ->
            dst [128, n] bf16, RoPE'd with cos/sin slices c, s [64, n]."""
            qe = psum[0:64, :]
            qo = psum[64:128, :]
            t0 = rope_pool.tile([64, n], F32, tag="rt0", name="rt0")
            t1 = rope_pool.tile([64, n], F32, tag="rt1", name="rt1")
            nc.vector.tensor_mul(t0[:], qe, c)
            nc.vector.tensor_mul(t1[:], qo, s)
            nc.vector.tensor_sub(dst[0:64, :], t0[:], t1[:])
            t2 = rope_pool.tile([64, n], F32, tag="rt0", name="rt2")
            t3 = rope_pool.tile([64, n], F32, tag="rt1", name="rt3")
            nc.vector.tensor_mul(t2[:], qe, s)
            nc.vector.tensor_mul(t3[:], qo, c)
            nc.vector.tensor_add(dst[64:128, :], t2[:], t3[:])

        # w2x: xtq/cosq/sinq, prefetched during W1, freed after C(b0).
        # w2w: wq group staging (2 heads), first groups prefetched in W1.
        with tc.tile_pool(name="w2x", bufs=1) as w2x, \
             tc.tile_pool(name="w2w", bufs=2) as w2w:

            xtq_sb = w2x.tile([128, KD, QTOK], BF16)
            cosq_sb = w2x.tile([64, QTOK], F32)
            sinq_sb = w2x.tile([64, QTOK], F32)

            wqg_tiles = {}

            def preload_wq(hg):
                wqg = w2w.tile([128, KD, 2 * HD], BF16, tag="wqg", name="wqg")
                nc.sync.dma_start(wqg[:], wq[hg])
                wqg_tiles[hg] = wqg

            # ========= W1: K/V projection (own 512 tokens, ALL kv heads) ===
            # xtq is loaded here (4MB, kg-interleaved with the first weight
            # group) and reused by W2 — x is read from HBM exactly once.
            with tc.tile_pool(name="p1", bufs=2) as p1, \
                 tc.tile_pool(name="p1ps", bufs=2, space="PSUM") as p1ps, \
                 tc.tile_pool(name="rope1", bufs=2) as rope1:

                wkv_tiles = {}

                def preload_wkv(hkv):
                    if hkv >= cfg.HKV:
                        return
                    wkg = p1.tile([128, KD, HD], BF16, tag="wkg", name="wkg")
                    wvg = p1.tile([128, KD, HD], BF16, tag="wvg", name="wvg")
                    nc.sync.dma_start(wkg[:], wk[hkv])
                    nc.sync.dma_start(wvg[:], wv[hkv])
                    wkv_tiles[hkv] = (wkg, wvg)

                # prologue: angles (tiny, needed by rope(0)), then
                # wkg0+xtq+wvg0 chunks interleaved; the fused K(0)+V(0)
                # below tracks the stream chunk-by-chunk so PE has ~16.5us
                # of work against the ~17.5us 4-stream DMA window
                wkg0 = p1.tile([128, KD, HD], BF16, tag="wkg", name="wkg")
                wvg0 = p1.tile([128, KD, HD], BF16, tag="wvg", name="wvg")
                for kg in range(0, KD, 4):
                    nc.sync.dma_start(wkg0[:, kg:kg + 4, :],
                                      wk[0, :, kg:kg + 4, :])
                    nc.sync.dma_start(xtq_sb[:, kg:kg + 4, :],
                                      xTq[:, kg:kg + 4, :])
                    nc.sync.dma_start(wvg0[:, kg:kg + 4, :],
                                      wv[0, :, kg:kg + 4, :])
                    if kg == 0:
                        # angles land after the first compute chunk; rope(0)
                        # only needs them at ~17us
                        nc.sync.dma_start(cosq_sb[:], cosq[:])
                        nc.sync.dma_start(sinq_sb[:], sinq[:])
                preload_wkv(1)

                # fused K(0)+V(0): per k-chunk matmuls in stream order
                pk0 = p1ps.tile([128, QTOK], F32, tag="pk", name="pk")
                pv0 = [
                    p1ps.tile([128, HD], F32, tag="pv0", bufs=4, name="pv0")
                    for _ in range(QTOK // 128)
                ]
                for kg in range(0, KD, 4):
                    for k in range(kg, kg + 4):
                        nc.tensor.matmul(
                            pk0[:], lhsT=wkg0[:, k, :], rhs=xtq_sb[:, k, :],
                            start=(k == 0), stop=(k == KD - 1))
                    for st in range(QTOK // 128):
                        for k in range(kg, kg + 4):
                            nc.tensor.matmul(
                                pv0[st][:],
                                lhsT=xtq_sb[:, k, st * 128:(st + 1) * 128],
                                rhs=wvg0[:, k, :],
                                start=(k == 0), stop=(k == KD - 1))
                kch = rope1.tile([128, QTOK], BF16, tag="kch", name="kch")
                rope_apply(rope1, kch[:], pk0, cosq_sb[:], sinq_sb[:], QTOK)
                nc.sync.dma_start(agin_k[0], kch[:])
                for st in range(QTOK // 128):
                    vch = rope1.tile([128, HD], BF16, tag="vch", name="vch")
                    nc.vector.tensor_copy(vch[:], pv0[st][:])
                    nc.sync.dma_start(
                        agin_v[0, st * 128:(st + 1) * 128, :], vch[:])
                preload_wkv(2)

                for hkv in range(1, cfg.HKV):
                    wkg, wvg = wkv_tiles.pop(hkv)
                    if hkv + 2 < cfg.HKV:
                        preload_wkv(hkv + 2)

                    def w1_k():
                        # K: psum [128, QTOK] -> rope -> agin_k[hkv]
                        pk = p1ps.tile([128, QTOK], F32, tag="pk", name="pk")
                        for k in range(KD):
                            nc.tensor.matmul(
                                pk[:], lhsT=wkg[:, k, :], rhs=xtq_sb[:, k, :],
                                start=(k == 0), stop=(k == KD - 1),
                            )
                        kch = rope1.tile([128, QTOK], BF16, tag="kch",
                                         name="kch")
                        rope_apply(rope1, kch[:], pk, cosq_sb[:], sinq_sb[:],
                                   QTOK)
                        nc.sync.dma_start(agin_k[hkv], kch[:])

                    if not _os.environ.get("KV2_VFIRST"):
                        w1_k()
                    # V: project directly in natural [t, hd] layout
                    for st in range(QTOK // 128):
                        pv = p1ps.tile([128, HD], F32, tag="pv", name="pv")
                        for k in range(KD):
                            nc.tensor.matmul(
                                pv[:],
                                lhsT=xtq_sb[:, k, st * 128:(st + 1) * 128],
                                rhs=wvg[:, k, :],
                                start=(k == 0), stop=(k == KD - 1),
                            )
                        vch = rope1.tile([128, HD], BF16, tag="vch",
                                         name="vch")
                        nc.vector.tensor_copy(vch[:], pv[:])
                        nc.sync.dma_start(
                            agin_v[hkv, st * 128:(st + 1) * 128, :], vch[:])
                    if _os.environ.get("KV2_VFIRST"):
                        w1_k()

                if collective:
                    nc.gpsimd.collective_compute(
                        "AllGather",
                        mybir.AluOpType.bypass,
                        replica_groups=[list(range(NC))],
                        ins=[ag_in.opt()],
                        outs=[ag_out.opt()],
                    )
                else:
                    nc.sync.dma_start(ag_out[0, 0:2048], ag_in[0:2048])

            # ============== attention machinery (per-phase pools) ==========
            # gathered views: kT [hkv, p, r, t'], v [hkv, p(token), r, so, hd]
            KVO = cfg.HKV * QTOK * HD
            agk_p = ag_out[:, 0:KVO].rearrange(
                "r (h p t) -> h p r t", h=cfg.HKV, p=128)
            agv_p = ag_out[:, KVO : 2 * KVO].rearrange(
                "r (h so p hd) -> h p r so hd", h=cfg.HKV, so=QTOK // 128,
                p=128)
            kv_seq = [(b, hkv) for b in range(B) for hkv in range(cfg.HKV)]

            def make_attn(kv, p2e, p2sb, p2ps, p2po, lo, hi):
                kv_tiles = {}

                def load_kv(i):
                    if i < lo or i >= hi:
                        return
                    b, hkv = kv_seq[i]
                    # global token r*TB + t' of batch b sits in core r's
                    # staging at column b*TB + t'
                    kT = kv.tile([128, T], BF16, tag="kT", name="kT")
                    for r in range(NC):
                        nc.sync.dma_start(
                            kT[:, r * TB:(r + 1) * TB],
                            agk_p[hkv, :, r, b * TB:(b + 1) * TB])
                    vt = kv.tile([128, NST, HD], BF16, tag="vt", name="vt")
                    spb = TB // 128          # s-tiles per core block (2)
                    for r in range(NC):
                        nc.sync.dma_start(
                            vt[:, r * spb:(r + 1) * spb, :],
                            agv_p[hkv, :, r, b * spb:(b + 1) * spb, :])
                    kv_tiles[(b, hkv)] = (kT, vt)

                pending = [None]

                def flush_pending():
                    th, pending[0] = pending[0], None
                    if th is not None:
                        th()

                def finalize(parts, po, b, h):
                    while len(parts) > 1:
                        lvl_dt = F32 if len(parts) == 2 else BF16
                        nxt = []
                        for x in range(0, len(parts) - 1, 2):
                            s = p2sb.tile([128, TB], lvl_dt, tag="dt",
                                          bufs=4, name="dt")
                            nc.vector.tensor_add(s[:], parts[x][:],
                                                 parts[x + 1][:])
                            nxt.append(s)
                        if len(parts) % 2:
                            nxt.append(parts[-1])
                        parts = nxt
                    dall = p2sb.tile([128, TB], F32, tag="dall",
                                     name="dall")
                    nc.gpsimd.partition_all_reduce(
                        dall[:], parts[0][:], channels=128,
                        reduce_op=bass_isa.ReduceOp.add)
                    rcp = p2sb.tile([128, TB], F32, tag="rcp", name="rcp")
                    nc.vector.reciprocal_approx_fast(rcp[:], dall[:])
                    nc.vector.tensor_mul(oT_sb[b][h][:], po[:], rcp[:])

                def attn_iter(i, fill):
                    b, hkv = kv_seq[i]
                    kT, vt = kv_tiles.pop((b, hkv))
                    for j in range(4):
                        h = hkv * 4 + j
                        if j == 1:
                            load_kv(i + 3)     # rolling prefetch (bufs=3)
                        qs = qT_sb[h][:, b * TB:(b + 1) * TB]
                        po = p2po.tile([128, TB], F32, tag="po", name="po")
                        parts = []
                        for sg in range(NSG):
                            ps = p2ps.tile([128, SGRP * TB], F32, tag="ps",
                                           name="ps")
                            for si in range(SGRP):
                                sidx = sg * SGRP + si
                                nc.tensor.matmul(
                                    ps[:, si * TB:(si + 1) * TB],
                                    lhsT=kT[:, sidx * 128:(sidx + 1) * 128],
                                    rhs=qs,
                                    start=True, stop=True,
                                )
                            e = p2e.tile([128, SGRP * TB], BF16, tag="e",
                                         name="e")
                            nc.scalar.activation(e[:], ps[:], AF.Exp,
                                                 scale=scale)
                            # software pipeline: PV for the PREVIOUS group
                            # is emitted only now, so its exp semaphore is
                            # long satisfied when PE reaches it (kills the
                            # per-group ACT->PE wait)
                            flush_pending()
                            fill.pop()      # PE filler while ScalarE exps

                            def pv_thunk(sg=sg, e=e, po=po, parts=parts,
                                         vt=vt, b=b, h=h):
                                for si in range(SGRP):
                                    sidx = sg * SGRP + si
                                    nc.tensor.matmul(
                                        po[:],
                                        lhsT=vt[:, sidx, :],
                                        rhs=e[:, si * TB:(si + 1) * TB],
                                        start=(sg == 0 and si == 0),
                                        stop=(sg == NSG - 1
                                              and si == SGRP - 1),
                                    )
                                d01 = p2sb.tile([128, 2 * TB], BF16,
                                                tag="d0", name="d01")
                                nc.vector.tensor_add(d01[:], e[:, 0:2 * TB],
                                                     e[:, 2 * TB:4 * TB])
                                dp = p2sb.tile([128, TB], BF16, tag="dp",
                                               bufs=NSG + 1, name="dp")
                                nc.vector.tensor_add(dp[:], d01[:, 0:TB],
                                                     d01[:, TB:2 * TB])
                                parts.append(dp)
                                if sg == NSG - 1:
                                    finalize(parts, po, b, h)

                            pending[0] = pv_thunk

                return load_kv, attn_iter, flush_pending

            # ============== W2: Q projection + C(b0) ======================
            with tc.tile_pool(name="w2ps", bufs=2, space="PSUM") as w2ps, \
                 tc.tile_pool(name="rope2", bufs=2) as rope2, \
                 tc.tile_pool(name="kva", bufs=3) as kva, \
                 tc.tile_pool(name="p2ea", bufs=3) as p2ea, \
                 tc.tile_pool(name="p2sba", bufs=2) as p2sba, \
                 tc.tile_pool(name="p2psa", bufs=2, space="PSUM") as p2psa, \
                 tc.tile_pool(name="p2poa", bufs=2, space="PSUM") as p2poa:

                load_kv, attn_iter, flush_attn = make_attn(
                    kva, p2ea, p2sba, p2psa, p2poa, 0, cfg.HKV)

                def q_pieces(hg):
                    """Emission pieces for one 2-head wq group (~0.9us PE
                    each): 4-matmul chunks, then the rope."""
                    if hg not in wqg_tiles:
                        preload_wq(hg)
                    wqg = wqg_tiles.pop(hg)
                    state = {}
                    pieces = []
                    for j in range(2):
                        h = hg * 2 + j

                        def mm(j, h, k0):
                            if k0 == 0:
                                state[h] = w2ps.tile([128, QTOK], F32,
                                                     tag="pq", name="pq")
                            pq = state[h]
                            for k in range(k0, k0 + 4):
                                nc.tensor.matmul(
                                    pq[:],
                                    lhsT=wqg[:, k, j * HD:(j + 1) * HD],
                                    rhs=xtq_sb[:, k, :],
                                    start=(k == 0), stop=(k == KD - 1),
                                )

                        pieces += [
                            (lambda j=j, h=h, k0=k0: mm(j, h, k0))
                            for k0 in range(0, KD, 4)
                        ]
                        pieces.append(
                            lambda h=h: rope_apply(
                                rope2, qT_sb[h][:], state.pop(h),
                                cosq_sb[:], sinq_sb[:], QTOK))
                    return pieces

                def q_group(hg):
                    for p in q_pieces(hg):
                        p()

                n_front = HG - cfg.DEFER
                q_group(0)
                preload_wq(1)
                q_group(1)
                for hg in range(2, n_front):
                    q_group(hg)
                    if hg in (4, 5, 6):
                        load_kv(hg - 4)

                for hg in range(n_front, HG):
                    preload_wq(hg)
                fill0 = Fill(rate=cfg.QRATE, credit=2.0)
                for hg in range(n_front, HG):
                    fill0.add_factory(lambda hg=hg: q_pieces(hg))
                for i in range(cfg.HKV):
                    attn_iter(i, fill0)
                flush_attn()
                fill0.flush()

        # =================== C(b1) + D: o_proj ========================
        DC = 256
        with tc.tile_pool(name="p3w", bufs=3) as p3w, \
             tc.tile_pool(name="p3o", bufs=3) as p3o, \
             tc.tile_pool(name="p3ps", bufs=2, space="PSUM") as p3ps, \
             tc.tile_pool(name="kvb", bufs=3) as kvb, \
             tc.tile_pool(name="p2eb", bufs=3) as p2eb, \
             tc.tile_pool(name="p2sbb", bufs=2) as p2sbb, \
             tc.tile_pool(name="p2psb", bufs=2, space="PSUM") as p2psb, \
             tc.tile_pool(name="p2pob", bufs=2, space="PSUM") as p2pob:

            load_kv, attn_iter, flush_attn = make_attn(
                kvb, p2eb, p2sbb, p2psb, p2pob, cfg.HKV, 2 * cfg.HKV)

            wo_tiles = {}

            def preload_wo(dch):
                if dch in wo_tiles or dch >= D // DC:
                    return
                wo_sb = p3w.tile([128, H, DC], BF16, tag="wo", name="wo_sb")
                nc.sync.dma_start(wo_sb[:], wo[dch])
                wo_tiles[dch] = wo_sb

            def d_chunk_pieces(dch, b, tt, nxt_dch=None):
                state = {}

                def mm(k0):
                    if k0 == 0:
                        if nxt_dch is not None:
                            preload_wo(nxt_dch)
                        state["pso"] = p3ps.tile([128, DC], F32, tag="pso",
                                                 name="pso")
                    pso = state["pso"]
                    wo_sb = wo_tiles[dch]
                    for k in range(k0, k0 + 4):
                        nc.tensor.matmul(
                            pso[:],
                            lhsT=oT_sb[b][k][:, tt * 128:(tt + 1) * 128],
                            rhs=wo_sb[:, k, :],
                            start=(k == 0), stop=(k == H - 1),
                        )

                def fin():
                    pso = state.pop("pso")
                    osb = p3o.tile([128, DC], F32, tag="osb", name="osb")
                    nc.vector.tensor_copy(osb[:], pso[:])
                    nc.sync.dma_start(
                        out[b * TB + tt * 128 : b * TB + (tt + 1) * 128,
                            dch * DC:(dch + 1) * DC],
                        osb[:],
                    )

                return [lambda k0=k0: mm(k0) for k0 in range(0, H, 4)] + [fin]

            def d_factory(dch):
                def make():
                    if dch not in wo_tiles:
                        preload_wo(dch)
                    return (d_chunk_pieces(dch, 0, 0, nxt_dch=dch + 1)
                            + d_chunk_pieces(dch, 0, 1))
                return make

            load_kv(cfg.HKV)
            load_kv(cfg.HKV + 1)
            fill1 = Fill(rate=cfg.DRATE, credit=2.0)
            preload_wo(0)
            for dch in range(cfg.DF_DCH):
                fill1.add_factory(d_factory(dch))
            load_kv(cfg.HKV + 2)
            for i in range(cfg.HKV, 2 * cfg.HKV):
                attn_iter(i, fill1)
            flush_attn()
            fill1.flush()

            # remaining o_proj: fresh dchs for both blocks, then the b=1
            # halves of the filler dchs (wo reloaded for those), with
            # one-ahead wo prefetch threaded through the first mm pieces
            for dch in range(cfg.DF_DCH):
                wo_tiles.pop(dch, None)      # stale filler staging refs
            fresh = [(dch, (0, 1)) for dch in range(cfg.DF_DCH, D // DC)]
            redo = [(dch, (1,)) for dch in range(cfg.DF_DCH)]
            work = []
            while fresh or redo:
                if fresh:
                    work.append(fresh.pop(0))
                if redo:
                    work.append(redo.pop(0))
            for i, (dch, bs) in enumerate(work):
                preload_wo(dch)
                if i + 2 < len(work):
                    preload_wo(work[i + 2][0])
                nxt = work[i + 1][0] if i + 1 < len(work) else None
                first = True
                for b in bs:
                    for tt in range(TB // 128):
                        for p in d_chunk_pieces(
                                dch, b, tt,
                                nxt_dch=(nxt if first else None)):
                            p()
                        first = False
                wo_tiles.pop(dch, None)


    nc.compile()
    return nc


# ------------------------------------------------------------------
# host-side input prep
# ------------------------------------------------------------------

def _rope_perm(n_heads_cols: int, HD: int) -> np.ndarray:
    """Column permutation: per head, evens first then odds."""
    idx = np.arange(n_heads_cols)
    h = idx // HD
    j = idx % HD
    old = np.where(j < HD // 2, 2 * j, 2 * (j - HD // 2) + 1)
    return h * HD + old


def make_in_maps(inputs: dict, cfg: Cfg = FULL):
    B, T, D, HD, NC = cfg.B, cfg.T, cfg.D, cfg.HD, cfg.NC
    TB = cfg.TB
    bf = ml_dtypes.bfloat16

    x = np.asarray(inputs["x"], np.float32).reshape(cfg.TOK, D)
    xT = np.ascontiguousarray(x.T).astype(bf)

    wq = np.asarray(inputs["wq"], np.float32)
    wk = np.asarray(inputs["wk"], np.float32)
    wv = np.asarray(inputs["wv"], np.float32)
    wo = np.asarray(inputs["wo"], np.float32)

    permq = _rope_perm(wq.shape[1], HD)
    permk = _rope_perm(wk.shape[1], HD)
    wq_p = np.ascontiguousarray(wq[:, permq]).astype(bf)
    wk_p = wk[:, permk].astype(bf)
    wv_b = wv.astype(bf)
    wo_b = np.ascontiguousarray(wo).astype(bf)

    cos = np.asarray(inputs["freqs_cos"], np.float32)   # [T, 64]
    sin = np.asarray(inputs["freqs_sin"], np.float32)
    cosT = np.ascontiguousarray(cos.T)
    sinT = np.ascontiguousarray(sin.T)

    KD, HG = D // 128, cfg.HG
    wk_c = np.ascontiguousarray(
        wk_p.reshape(KD, 128, cfg.HKV, HD).transpose(2, 1, 0, 3))
    wv_c = np.ascontiguousarray(
        wv_b.reshape(KD, 128, cfg.HKV, HD).transpose(2, 1, 0, 3))
    wq_c = np.ascontiguousarray(
        wq_p.reshape(KD, 128, HG, 2 * HD).transpose(2, 1, 0, 3))
    wo_c = np.ascontiguousarray(
        wo_b.reshape(cfg.H, 128, D // 256, 256).transpose(2, 1, 0, 3))
    in_maps = []
    for c in range(NC):
        tsl = slice(c * TB, (c + 1) * TB)          # own positions per batch
        xTq = np.concatenate(
            [xT[:, b * T + c * TB : b * T + (c + 1) * TB] for b in range(B)],
            axis=1,
        )
        xTq = xTq.reshape(KD, 128, B * TB).transpose(1, 0, 2)
        cq = np.concatenate([cosT[:, tsl]] * B, axis=1)
        sq = np.concatenate([sinT[:, tsl]] * B, axis=1)
        in_maps.append({
            "xTq": np.ascontiguousarray(xTq),
            "wq": wq_c,
            "wk": wk_c,
            "wv": wv_c,
            "wo": wo_c,
            "cosq": np.ascontiguousarray(cq),
            "sinq": np.ascontiguousarray(sq),
        })
    return in_maps


_CACHE: dict = {}


def kernel(**inputs) -> np.ndarray:
    cfg = FULL
    sp = inputs.get("start_pos", 0)
    sp = int(np.asarray(sp).reshape(-1)[0]) if np.asarray(sp).size else 0
    assert sp == 0, f"kernel only supports start_pos=0, got {sp}"

    from concourse.bass_utils import run_bass_kernel_spmd

    if "nc" not in _CACHE:
        _CACHE["nc"] = build_nc(cfg)
    nc = _CACHE["nc"]

    in_maps = make_in_maps(inputs, cfg)
    res = run_bass_kernel_spmd(nc, in_maps, list(range(cfg.NC)))
    full = np.empty((cfg.TOK, cfg.D), np.float32)
    TB = cfg.TB
    for c in range(cfg.NC):
        o = res.results[c]["out"]
        for b in range(cfg.B):
            full[b * cfg.T + c * TB : b * cfg.T + (c + 1) * TB] = \
                o[b * TB:(b + 1) * TB]
    return full.reshape(cfg.B, cfg.T, cfg.D)


if __name__ == "__main__":
    nc = build_nc()
    n = sum(len(bb.instructions) for bb in nc.m.functions[0].blocks)
    print("built", n, "instructions")

